# revision 26
# baseline (speedup 1.0000x reference)
"""Trainium2 Bass kernel for nn_DGLRegressor (4-layer GCN + mean-pool + MLP head).

Math: each GraphConv layer l computes, per dst node n,
    agg_n = sum_{e: dst=n} p_l[src_e],    p_l = h_l' @ W_l  (h' = h * isr_out)
    h_{l+1}' = isr_out * relu(agg * isr_in + b_l)           (l < 3)
    h_4      = relu(agg * isr_in + b_4)                     (l = 3, mean-pooled)
with isr_in applied as a per-column (per-node) DVE scale on the PSUM agg, and
the relu bias as a per-partition ACT bias.

Sharding (8 cores): nodes dealt to (core, tile, slot) by a global in-degree
snake over all 800 tiles (balances per-tile edge counts across cores, which
sets the static gather sizes). 100 dst tiles of 128 nodes per core, grouped
into 25 supertiles of 4 tiles sharing one [128, 512] PSUM bank.

Per layer: p tables are fp8-AllGathered piecewise (4 sub-slices of 32/32/32/4
tiles; the tiny last sub shortens the exposed inter-layer collective tail),
then cast-DMA'd to fp16 chunk tables (<=32768 rows each, int16-indexable).
Per (supertile, chunk): ONE bulk dma_gather (SWDGE) of all 4 tiles' rows
(exact per-(tile,chunk) row counts; trailing pad idx -1 = skipped), amortizing
the ~1us per-call SWDGE fixed cost ~16x vs per-(tile,chunk) gathers. Tile
boundaries inside a merged gather fall mid-block; boundary blocks appear once
per adjacent tile in the eq (one-hot) array with rows of other tiles masked
(dst-label 200 never matches the 0..127 iota), so no 128-row padding is
needed. One wide DVE is_equal per supertile builds all eq blocks; matmuls
agg^T += msg^T @ M accumulate per dst tile in PSUM.
"""

import os
import numpy as np

import concourse.bacc as bacc
import concourse.bass as bass
import concourse.tile as tile
import concourse.mybir as mybir
import concourse.bass_utils as bass_utils

F16 = mybir.dt.float16
F32 = mybir.dt.float32
F8 = mybir.dt.float8e4
I16 = mybir.dt.int16

D = 128
N_CORES = 8
ST = 4  # tiles per supertile
N_REAL = 100000
TILES = 100                 # per core
STILES = TILES // ST        # 25
OWN = TILES * D             # 12800 rows per core (incl 300 pads)
SUB_TILES = (32, 32, 20, 16)  # tiles per sub-slice / chunk (late subs small:
SUB_BASE = (0, 32, 64, 84)    # their AllGathers fire last and set the tail)
N_PAD = N_CORES * OWN - N_REAL

LAST_PERF = {}


# --------------------------------------------------------------------------
# host-side structure preprocessing (graph only: routing, layout, metadata)
# --------------------------------------------------------------------------

def _preprocess(x, src, dst):
    src = np.asarray(src).astype(np.int64)
    dst = np.asarray(dst).astype(np.int64)
    deg_out = np.bincount(src, minlength=N_REAL).astype(np.float32)
    deg_in = np.bincount(dst, minlength=N_REAL).astype(np.float32)
    isr_out = 1.0 / np.sqrt(np.maximum(deg_out, 1.0))
    isr_in = 1.0 / np.sqrt(np.maximum(deg_in, 1.0))

    # node -> (core, tile, slot): global snake deal by in-degree over all
    # 800 (core, tile) pairs; balances per-(tile) edge counts across cores.
    n_gtiles = N_CORES * TILES
    order = np.argsort(-deg_in, kind="stable")
    rank = np.empty(N_REAL, np.int64)
    rank[order] = np.arange(N_REAL)
    rnd = rank // n_gtiles           # 0..124 (exactly 125 rounds)
    idx = rank % n_gtiles
    gt = np.where(rnd % 2 == 0, idx, n_gtiles - 1 - idx)
    core_of = gt // TILES
    tile_of = gt % TILES
    slot_of = rnd                    # 0..124; slots 125-127 are pads
    assert slot_of.max() < D

    sub_of_tile = np.zeros(TILES, np.int64)
    for s in range(4):
        sub_of_tile[SUB_BASE[s]:SUB_BASE[s] + SUB_TILES[s]] = s
    # chunk-table row for each node (src side). Rows are interleaved
    # [core, supertile-group, slot, tile-in-st] so that table writers (stage-A
    # slabs and per-supertile po stores, both with partition=slot) hit 4
    # CONSECUTIVE rows per partition = 1KB contiguous DMA descriptors.
    s_of = sub_of_tile[tile_of]
    u_of = tile_of - np.asarray(SUB_BASE)[s_of]
    sub_rows = np.asarray([t * D for t in SUB_TILES])
    rel_row = (core_of * sub_rows[s_of] + (u_of // ST) * (ST * D)
               + slot_of * ST + (u_of % ST))
    assert rel_row.max() < 32768

    # per-(core, tile, chunk) counts -> static r_tc = max over cores (exact)
    e_core = core_of[dst]
    e_tile = tile_of[dst]
    e_chunk = s_of[src]
    counts = np.zeros((N_CORES, TILES, 4), np.int64)
    np.add.at(counts, (e_core, e_tile, e_chunk), 1)
    r_tc = counts.max(axis=0)        # [TILES, 4]

    # region (st, c): all 4 tiles' chunk-c rows in one gather
    a_off = np.zeros((STILES, 4, ST), np.int64)     # row of tile u in region
    R_raw = np.zeros((STILES, 4), np.int64)
    for si in range(STILES):
        for c in range(4):
            acc = 0
            for u in range(ST):
                a_off[si, c, u] = acc
                acc += r_tc[si * ST + u, c]
            R_raw[si, c] = acc
    R_pad = (R_raw + 15) // 16 * 16                 # num_idxs (x16)
    B_sc = (R_pad + 127) // 128                     # msg blocks per region
    # idx col offsets (region-major: st, then c)
    s_off = np.zeros((STILES, 4), np.int64)
    np.cumsum((R_pad // 16).ravel()[:-1], out=s_off.ravel()[1:])
    total_s = int((R_pad // 16).sum())
    row_base = s_off * 16                           # region row base, global
    # msg block offset of region c within supertile buffer
    moff = np.zeros((STILES, 4), np.int64)
    for si in range(STILES):
        moff[si] = np.cumsum(np.concatenate([[0], B_sc[si, :3]]))
    B_st = B_sc.sum(axis=1)
    BSTMAX = int(B_st.max())

    # eq blocks: per (st, c, u): msg blocks [mb0, mb1) of tile u's segment
    mb0 = np.zeros((STILES, 4, ST), np.int64)
    nmb = np.zeros((STILES, 4, ST), np.int64)
    for si in range(STILES):
        for c in range(4):
            for u in range(ST):
                a = a_off[si, c, u]
                r = r_tc[si * ST + u, c]
                if r == 0:
                    continue
                mb0[si, c, u] = a // 128
                nmb[si, c, u] = (a + r + 127) // 128 - a // 128
    # eq col layout per supertile: order (c, u, j); global dstl col = D0 + ...
    eoff = np.zeros((STILES, 4, ST), np.int64)
    NEQ = np.zeros(STILES, np.int64)
    for si in range(STILES):
        acc = 0
        for c in range(4):
            for u in range(ST):
                eoff[si, c, u] = acc
                acc += nmb[si, c, u]
        NEQ[si] = acc
    D0 = np.zeros(STILES, np.int64)
    np.cumsum(NEQ[:-1], out=D0[1:])
    total_dstl = int(NEQ.sum())
    EQMAX = int(NEQ.max())

    meta = dict(r_tc=r_tc, a_off=a_off, R_raw=R_raw, R_pad=R_pad, B_sc=B_sc,
                s_off=s_off, total_s=total_s, moff=moff, BSTMAX=BSTMAX,
                mb0=mb0, nmb=nmb, eoff=eoff, NEQ=NEQ, D0=D0,
                total_dstl=total_dstl, EQMAX=EQMAX)

    # ---- per-core data arrays ----
    col = tile_of * D + slot_of     # own-column of each node on its core
    xt = np.asarray(x, dtype=np.float32)
    # full x' in table-column order (chunk-major, rel_row within chunk):
    # every core computes the whole p1 = x'@W1 table locally (no stage-A AG)
    csize = tuple(t * D * N_CORES for t in SUB_TILES)
    coff = np.cumsum([0] + list(csize[:3]))
    # xTf column order is the stage-A compute order: slab k's sub-matmul j,
    # partition p computes the node stored at table row k*512 + p*4 + j
    k_, r_ = rel_row // (ST * D), rel_row % (ST * D)
    xcol = coff[s_of] + k_ * (ST * D) + (r_ % ST) * D + r_ // ST
    xTf = np.zeros((D, sum(csize)), np.float16)
    xTf[:, xcol] = (xt * isr_out[:, None]).T.astype(np.float16)
    per_core = []
    # per-edge static placement (same for every core's own edges)
    e_st = e_tile // ST
    e_u = e_tile % ST
    for cc in range(N_CORES):
        m = e_core == cc
        es, ed = src[m], dst[m]
        est, eu, ech = e_st[m], e_u[m], e_chunk[m]
        key = (est * 4 + ech) * ST + eu
        o = np.argsort(key, kind="stable")
        es, ed, est, eu, ech = es[o], ed[o], est[o], eu[o], ech[o]
        key = key[o]
        grp_start = np.searchsorted(key, np.arange(STILES * 4 * ST))
        erank = np.arange(len(key)) - grp_start[key]
        # global gather row of each edge
        grow = row_base[est, ech] + a_off[est, ech, eu] + erank

        idx_flat = np.zeros(total_s * 16, np.int16)  # pads gather row 0 (masked)
        idx_flat[grow] = rel_row[es].astype(np.int16)
        # core pads: rows [a+ct, a+r) per (t,c) -> gather row 0 (masked)
        ct = counts[cc]             # [TILES, 4]
        for si in range(STILES):
            for c in range(4):
                for u in range(ST):
                    t = si * ST + u
                    c0, r = int(ct[t, c]), int(r_tc[t, c])
                    if c0 < r:
                        b0 = row_base[si, c] + a_off[si, c, u]
                        idx_flat[b0 + c0:b0 + r] = 0
        idx16 = idx_flat.reshape(total_s, 16).T.copy()

        dstl = np.full((128, total_dstl), 200.0, np.float16)
        # eq col of each edge: D0[st] + eoff[st,c,u] + (rowinregion//128 - mb0)
        rin = a_off[est, ech, eu] + erank
        g = D0[est] + eoff[est, ech, eu] + (rin // 128 - mb0[est, ech, eu])
        p = rin % 128
        dstl[p, g] = slot_of[ed].astype(np.float16)

        nodes = np.nonzero(core_of == cc)[0]
        ncol = col[nodes]
        inB = np.ones(OWN, np.float32)
        inB[ncol] = isr_in[nodes]
        outB = np.ones(OWN, np.float32)
        outB[ncol] = isr_out[nodes]
        per_core.append(dict(
            idx=np.tile(idx16, (8, 1)), dstl=dstl,
            isrinB=np.tile(inB.astype(np.float16)[None, :], (D, 1)),
            isroutB=np.tile(outB.astype(np.float16)[None, :], (D, 1)),
        ))

    meta["iota"] = np.tile(np.arange(D, dtype=np.float16)[None, :], (D, 1))
    meta["xTf"] = xTf
    return meta, per_core


# --------------------------------------------------------------------------
# device program
# --------------------------------------------------------------------------

def _build(meta):
    r_tc = meta["r_tc"]
    R_pad, B_sc = meta["R_pad"], meta["B_sc"]
    s_off, total_s = meta["s_off"], meta["total_s"]
    moff, BSTMAX = meta["moff"], meta["BSTMAX"]
    mb0, nmb, eoff = meta["mb0"], meta["nmb"], meta["eoff"]
    NEQ, D0, total_dstl = meta["NEQ"], meta["D0"], meta["total_dstl"]
    EQMAX = meta["EQMAX"]
    csize = tuple(t * D * N_CORES for t in SUB_TILES)

    # SWDGE queues: Tile assigns Pool DMA instructions to 8 DMASW semaphore
    # lanes round-robin in emission order, and a semaphore may only ever be
    # updated from ONE SWDGE queue (ucode shadow-sem accounting). The casts
    # (gpsimd.dma_start) are pinned to queue 0, so the lane->queue map must
    # send every cast-hosting lane to 0. Emission counts are arranged so all
    # casts land on lanes 4-7 (layer 0 = 100 pool-DMAs, layers 1-3 = 104
    # each), giving the consistent map below. GCN_NQ=1 forces single-queue.
    # (4-queue lane-consistent mapping fails: the Tile scheduler reorders
    # instructions before DMASW sem assignment, so emission-order arithmetic
    # cannot keep a semaphore on one queue. Single queue is always safe.)
    n_queues = int(os.environ.get("GCN_NQ", "1"))
    QMAP = [0, 1 % n_queues, 2 % n_queues, 3 % n_queues, 0, 0, 0, 0]
    nc = bacc.Bacc("TRN2", target_bir_lowering=False, debug=False,
                   num_devices=N_CORES, num_swdge_queues=n_queues)

    # inputs
    xTf = nc.dram_tensor("xTf", [D, sum(csize)], F16, kind="ExternalInput").ap()
    idx_t = nc.dram_tensor("idx", [128, total_s], I16, kind="ExternalInput").ap()
    dstl_t = nc.dram_tensor("dstl", [128, total_dstl], F16, kind="ExternalInput").ap()
    iota_t = nc.dram_tensor("iota", [D, D], F16, kind="ExternalInput").ap()
    isrin_t = nc.dram_tensor("isrinB", [D, OWN], F16, kind="ExternalInput").ap()
    isrout_t = nc.dram_tensor("isroutB", [D, OWN], F16, kind="ExternalInput").ap()
    W16 = [nc.dram_tensor(f"W{i+1}", [D, D], F16, kind="ExternalInput").ap() for i in range(4)]
    Bv = [nc.dram_tensor(f"b{i+1}", [D, 1], F32, kind="ExternalInput").ap() for i in range(4)]
    Wl1 = nc.dram_tensor("Wl1", [D, D], F32, kind="ExternalInput").ap()
    Wl2 = nc.dram_tensor("Wl2", [D, D], F32, kind="ExternalInput").ap()
    Wo = nc.dram_tensor("Wo", [D, 1], F32, kind="ExternalInput").ap()
    bl1 = nc.dram_tensor("bl1", [D, 1], F32, kind="ExternalInput").ap()
    bl2 = nc.dram_tensor("bl2", [D, 1], F32, kind="ExternalInput").ap()
    bo = nc.dram_tensor("bo", [D, 1], F32, kind="ExternalInput").ap()
    out_t = nc.dram_tensor("out", [D, 1], F32, kind="ExternalOutput").ap()

    # internal DRAM: per (layer, sub): own piece (fp8), AG out (fp8, Shared),
    # fp16 gather table (cast-DMA rebuilt per sub). Layer 0's tables are
    # written directly by stage A (no AG/cast), so pown/pfull8 start at l=1.
    pown = [None] + [[nc.dram_tensor(f"pown{l}_{s}", [SUB_TILES[s] * D, D], F8)
                      for s in range(4)] for l in range(1, 4)]
    pfull8 = [None] + [[nc.dram_tensor(f"pfull8{l}_{s}", [csize[s], D], F8,
                                       addr_space="Shared")
                        for s in range(4)] for l in range(1, 4)]
    pfull = [[nc.dram_tensor(f"pfull{l}_{s}", [csize[s], D], F16)
              for s in range(4)] for l in range(4)]
    pool_b = [nc.dram_tensor("pool_in", [D, 1], F32),
              nc.dram_tensor("pool_out", [D, 1], F32)]

    RG = [list(range(N_CORES))]
    qctr = [0]

    with tile.TileContext(nc) as tc:
        with tc.tile_pool(name="const", bufs=1) as constp, \
             tc.tile_pool(name="eqm", bufs=2) as eqp, \
             tc.tile_pool(name="psA", bufs=3, space="PSUM") as psA, \
             tc.tile_pool(name="psB", bufs=3, space="PSUM") as psB, \
             tc.tile_pool(name="psH", bufs=1, space="PSUM") as psH, \
             tc.tile_pool(name="hpo", bufs=2) as hp:

            # resident constants / metadata
            idx_sb = constp.tile([128, total_s], I16)
            nc.sync.dma_start(out=idx_sb[:], in_=idx_t[:])
            dstl_sb = constp.tile([128, total_dstl], F16)
            nc.sync.dma_start(out=dstl_sb[:], in_=dstl_t[:])
            iota_sb = constp.tile([D, 1, D], F16)
            nc.sync.dma_start(out=iota_sb[:, 0, :], in_=iota_t[:])
            isrin_sb = constp.tile([D, OWN], F16)
            nc.sync.dma_start(out=isrin_sb[:], in_=isrin_t[:])
            isrout_sb = constp.tile([D, OWN], F16)
            nc.sync.dma_start(out=isrout_sb[:], in_=isrout_t[:])
            W_sb = []
            for i in range(4):
                w = constp.tile([D, D], F16, tag=f"W{i}")
                nc.sync.dma_start(out=w[:], in_=W16[i][:])
                W_sb.append(w)
            b_sb = []
            for i in range(4):
                b = constp.tile([D, 1], F32, tag=f"b{i}")
                nc.sync.dma_start(out=b[:], in_=Bv[i][:])
                b_sb.append(b)
            Wl1_sb = constp.tile([D, D], F32); nc.sync.dma_start(out=Wl1_sb[:], in_=Wl1[:])
            Wl2_sb = constp.tile([D, D], F32); nc.sync.dma_start(out=Wl2_sb[:], in_=Wl2[:])
            Wo_sb = constp.tile([D, 1], F32); nc.sync.dma_start(out=Wo_sb[:], in_=Wo[:])
            bl1_sb = constp.tile([D, 1], F32); nc.sync.dma_start(out=bl1_sb[:], in_=bl1[:])
            bl2_sb = constp.tile([D, 1], F32); nc.sync.dma_start(out=bl2_sb[:], in_=bl2[:])
            bo_sb = constp.tile([D, 1], F32); nc.sync.dma_start(out=bo_sb[:], in_=bo[:])
            pool_parts = constp.tile([D, STILES], F32)

            # persistent msg ring (2 supertiles), memset once: stale bytes
            # stay finite and are masked by eq=0
            msgs = []
            for r in range(2):
                mt = constp.tile([128, BSTMAX, D], F16, tag=f"msgr{r}")
                nc.vector.memset(mt[:, :, :], 0)
                msgs.append(mt)

            def do_allgather(l, s):
                nc.gpsimd.collective_compute(
                    "AllGather", mybir.AluOpType.bypass, replica_groups=RG,
                    ins=[pown[l][s].ap().opt()], outs=[pfull8[l][s].ap().opt()])

            def do_casts(l):
                # fp8 -> fp16 table rebuild; emitted at the START of layer l so
                # the cast's AG-wait doesn't block Pool.SEQ mid-way through the
                # previous layer's gather stream
                for s in range(4):
                    assert qctr[0] % 8 in (4, 5, 6, 7), qctr[0]
                    nc.gpsimd.dma_start(out=pfull[l][s].ap()[:, :],
                                        in_=pfull8[l][s].ap()[:, :])
                    qctr[0] += 1

            def store_po(l, st_i, po):
                # write the supertile's 4 own tiles into this layer's pown
                # piece (one store: rows interleaved [slot, tile] -> 1KB per
                # partition); fire the sub's AllGather on its last supertile
                t0 = st_i * ST
                s = 0
                while t0 >= SUB_BASE[s] + SUB_TILES[s]:
                    s += 1
                u0 = t0 - SUB_BASE[s]
                g0 = (u0 // ST) * (ST * D)
                nc.sync.dma_start(
                    out=pown[l][s][g0:g0 + ST * D, :]
                        .rearrange("(p j) f -> p j f", j=ST),
                    in_=po[:, :].rearrange("p (j f) -> p j f", j=ST))
                if u0 + ST == SUB_TILES[s]:
                    do_allgather(l, s)

            def post_supertile(l, st_i, agg):
                c0 = st_i * ST * D
                # per-node isr_in scale on the agg (column scale), PSUM->SBUF
                sc = hp.tile([D, ST * D], F32, tag="sc")
                nc.vector.tensor_tensor(
                    out=sc[:, :], in0=agg[:, :],
                    in1=isrin_sb[:, c0:c0 + ST * D],
                    op=mybir.AluOpType.mult)
                h = hp.tile([D, ST * D], F16, tag="h")
                nc.scalar.activation(h[:, :], sc[:, :],
                                     mybir.ActivationFunctionType.Relu,
                                     bias=b_sb[l][:], scale=1.0)
                if l == 3:
                    nc.vector.tensor_reduce(out=pool_parts[:, st_i:st_i + 1],
                                            in_=h[:, :],
                                            axis=mybir.AxisListType.X,
                                            op=mybir.AluOpType.add)
                    return
                nc.vector.tensor_tensor(
                    out=h[:, :], in0=h[:, :],
                    in1=isrout_sb[:, c0:c0 + ST * D],
                    op=mybir.AluOpType.mult)
                pp = psB.tile([D, ST * D], F32, tag="pps")
                for u in range(ST):
                    nc.tensor.matmul(out=pp[:, u * D:(u + 1) * D],
                                     lhsT=h[:, u * D:(u + 1) * D],
                                     rhs=W_sb[l + 1][:], start=True, stop=True)
                po = hp.tile([D, ST * D], F8, tag="po")
                nc.scalar.activation(po[:, :], pp[:, :],
                                     mybir.ActivationFunctionType.Copy)
                store_po(l + 1, st_i, po)

            # ---- stage A: every core computes the FULL p1 = x'@W1 table
            # locally, chunk-major (no collective; chunk-c tables complete in
            # order so layer 0's chunk-c gathers start while c+1 computes).
            # 4 slabs (2048 rows) per load/store to amortize HWDGE fixed cost.
            coff = np.cumsum([0] + list(csize[:3]))
            QS = 4
            GW = QS * ST * D   # 2048 rows per group
            for c in range(4):
                assert csize[c] % GW == 0
                for k in range(csize[c] // GW):
                    a = int(coff[c]) + k * GW
                    xs = hp.tile([D, GW], F16, tag="xsl")
                    nc.sync.dma_start(out=xs[:, :], in_=xTf[:, a:a + GW])
                    pt = hp.tile([D, GW], F16, tag="pt")
                    for q in range(QS):
                        pp = psB.tile([D, ST * D], F32, tag="pps")
                        for u in range(ST):
                            nc.tensor.matmul(
                                out=pp[:, u * D:(u + 1) * D],
                                lhsT=xs[:, q * ST * D + u * D:
                                        q * ST * D + (u + 1) * D],
                                rhs=W_sb[0][:], start=True, stop=True)
                        nc.scalar.activation(
                            pt[:, q * ST * D:(q + 1) * ST * D], pp[:, :],
                            mybir.ActivationFunctionType.Copy)
                    r0 = k * GW
                    nc.sync.dma_start(
                        out=pfull[0][c][r0:r0 + GW, :]
                            .rearrange("(q p j) f -> p q (j f)", q=QS, j=ST),
                        in_=pt[:, :].rearrange("p (q x) -> p q x", q=QS))

            # ---- stage B: 4 conv layers ----
            for l in range(4):
                if l:
                    do_casts(l)
                for st_i in range(STILES):
                    msg = msgs[st_i % 2]
                    for c in range(4):
                        R = int(R_pad[st_i, c])
                        assert R > 0
                        mo = int(moff[st_i, c])
                        # single_packet=False: the gather ucode puts ALL descs
                        # of a call into one packet when True, but packets cap
                        # at 64 descs (= 1008 rows) in HW — larger hangs SDMA.
                        nc.gpsimd.dma_gather(
                            out_ap=msg[:, mo:mo + int(B_sc[st_i, c]), :],
                            in_ap=pfull[l][c].ap()[:, :],
                            idxs_ap=idx_sb[:, int(s_off[st_i, c]):
                                           int(s_off[st_i, c]) + R // 16],
                            num_idxs=R, num_idxs_reg=R, elem_size=D,
                            single_packet=False,
                            queue_num=QMAP[qctr[0] % 8])
                        qctr[0] += 1
                    neq = int(NEQ[st_i])
                    eq = eqp.tile([128, EQMAX, D], F16, tag="eq")
                    nc.vector.tensor_tensor(
                        out=eq[:, :neq, :],
                        in0=dstl_sb[:, int(D0[st_i]):int(D0[st_i]) + neq, None]
                            .to_broadcast([128, neq, D]),
                        in1=iota_sb[:, 0:1, :].to_broadcast([128, neq, D]),
                        op=mybir.AluOpType.is_equal)
                    agg = psA.tile([D, ST * D], F32, tag="agg")
                    for u in range(ST):
                        first = True
                        mms = []
                        for c in range(4):
                            for j in range(int(nmb[st_i, c, u])):
                                mms.append((int(moff[st_i, c]) + int(mb0[st_i, c, u]) + j,
                                            int(eoff[st_i, c, u]) + j))
                        for k, (mb, eb) in enumerate(mms):
                            nc.tensor.matmul(out=agg[:, u * D:(u + 1) * D],
                                             lhsT=msg[:, mb, :],
                                             rhs=eq[:, eb, :],
                                             start=(k == 0),
                                             stop=(k == len(mms) - 1))
                    post_supertile(l, st_i, agg)

            # ---- pooling + head (replicated on every core) ----
            psum_pool = constp.tile([D, 1], F32)
            nc.vector.tensor_reduce(out=psum_pool[:], in_=pool_parts[:],
                                    axis=mybir.AxisListType.X, op=mybir.AluOpType.add)
            nc.sync.dma_start(out=pool_b[0].ap()[:, :], in_=psum_pool[:])
            nc.gpsimd.collective_compute(
                "AllReduce", mybir.AluOpType.add, replica_groups=RG,
                ins=[pool_b[0].ap().opt()], outs=[pool_b[1].ap().opt()])
            sum_all = constp.tile([D, 1], F32)
            nc.sync.dma_start(out=sum_all[:], in_=pool_b[1].ap()[:, :])
            # hg = (sum_all - n_pad*relu(b4)) / n_real
            relu_b4 = constp.tile([D, 1], F32)
            nc.scalar.activation(relu_b4[:], b_sb[3][:], mybir.ActivationFunctionType.Relu)
            corr = constp.tile([D, 1], F32)
            nc.vector.tensor_scalar_mul(out=corr[:], in0=relu_b4[:], scalar1=-float(N_PAD))
            hg = constp.tile([D, 1], F32)
            nc.vector.tensor_tensor(out=hg[:], in0=sum_all[:], in1=corr[:],
                                    op=mybir.AluOpType.add)
            nc.vector.tensor_scalar_mul(out=hg[:], in0=hg[:], scalar1=1.0 / N_REAL)

            ps1 = psH.tile([D, 1], F32, tag="head")
            nc.tensor.matmul(out=ps1[:], lhsT=Wl1_sb[:], rhs=hg[:], start=True, stop=True)
            hg1 = constp.tile([D, 1], F32)
            nc.scalar.activation(hg1[:], ps1[:], mybir.ActivationFunctionType.Relu,
                                 bias=bl1_sb[:], scale=1.0)
            ps2 = psH.tile([D, 1], F32, tag="head")
            nc.tensor.matmul(out=ps2[:], lhsT=Wl2_sb[:], rhs=hg1[:], start=True, stop=True)
            hg2 = constp.tile([D, 1], F32)
            nc.scalar.activation(hg2[:], ps2[:], mybir.ActivationFunctionType.Relu,
                                 bias=bl2_sb[:], scale=1.0)
            ps3 = psH.tile([1, 1], F32, tag="head1")
            nc.tensor.matmul(out=ps3[:], lhsT=Wo_sb[:], rhs=hg2[:], start=True, stop=True)
            res = constp.tile([1, 1], F32)
            nc.vector.tensor_tensor(out=res[:], in0=ps3[:], in1=bo_sb[0:1, :],
                                    op=mybir.AluOpType.add)
            nc.sync.dma_start(out=out_t[0:1, :], in_=res[:])

    nc.compile()
    return nc


# --------------------------------------------------------------------------
# entry point
# --------------------------------------------------------------------------

def _prepare(x, src, dst, W1, b1, W2, b2, W3, b3, W4, b4, Wl1, bl1, Wl2, bl2, Wo, bo):
    meta, per_core = _preprocess(x, src, dst)
    nc = _build(meta)

    Ws = [W1, W2, W3, W4]
    bs = [b1, b2, b3, b4]
    common = {}
    for i in range(4):
        common[f"W{i+1}"] = np.asarray(Ws[i], np.float32).astype(np.float16)
        common[f"b{i+1}"] = np.asarray(bs[i], np.float32).reshape(D, 1)
    common["Wl1"] = np.asarray(Wl1, np.float32)
    common["Wl2"] = np.asarray(Wl2, np.float32)
    common["Wo"] = np.asarray(Wo, np.float32).reshape(D, 1)
    common["bl1"] = np.asarray(bl1, np.float32).reshape(D, 1)
    common["bl2"] = np.asarray(bl2, np.float32).reshape(D, 1)
    common["bo"] = np.tile(np.asarray(bo, np.float32).reshape(1, 1), (D, 1))
    common["iota"] = meta["iota"]

    common["xTf"] = meta["xTf"]
    in_maps = []
    for c in range(N_CORES):
        m = dict(common)
        for k in ("idx", "dstl", "isrinB", "isroutB"):
            m[k] = per_core[c][k]
        in_maps.append(m)
    return nc, in_maps


def _run(x, src, dst, *args):
    nc, in_maps = _prepare(x, src, dst, *args)
    try:
        res = bass_utils.run_bass_kernel_spmd(
            nc, in_maps, core_ids=list(range(N_CORES)),
            trace=bool(int(os.environ.get("GCN_TRACE", "1"))))
    except ModuleNotFoundError:
        res = bass_utils.run_bass_kernel_spmd(
            nc, in_maps, core_ids=list(range(N_CORES)), trace=False)
    LAST_PERF.clear()
    LAST_PERF["exec_time_ns"] = res.exec_time_ns
    LAST_PERF["trace"] = res.instructions_and_trace[1] if res.instructions_and_trace else None
    return res.results[0]["out"][0:1, :].astype(np.float32)


def kernel(x, src, dst, W1, b1, W2, b2, W3, b3, W4, b4, Wl1, bl1, Wl2, bl2, Wo, bo):
    return _run(x, src, dst, W1, b1, W2, b2, W3, b3, W4, b4,
                Wl1, bl1, Wl2, bl2, Wo, bo)
